# revision 1
# baseline (speedup 1.0000x reference)
"""Causal multi-head attention (B=4, N=2048, D=768, H=12) on 8 TRN2 cores.

Sharding: data-parallel over batch (4) x 2-way query-row interleave
(core parity p takes rows p::2 of its batch). Every core runs the SAME
program: row interleaving makes the causal structure identical across
cores; the +-1-element diagonal difference is carried as input data
(bf16 mask-pattern tiles applied via an accumulating matmul).

Per core, fully on-device, no collectives:
  qT = Wq^T xq^T   kT = Wk^T x^T   v = x Wv   (x^T supplied by host)
  per (head, j-tile): sT = kT_j^T qT (+ causal bias); eT = exp(sT/8)
  oT[65, i] += v_aug_j^T eT   (65th v column = ones -> softmax denoms)
  attn^T = oT[0:64] * (1/denom)   out = attn^T^T Wo
"""

import numpy as np

B, N, D, H = 4, 2048, 768, 12
DH = D // H          # 64
NL = N // 2          # 1024 local query rows per core
KC = D // 128        # 6 contraction chunks
FT = D // 128        # 6 feature tiles (2 heads each)
JT = N // 128        # 16 key tiles
NEG = -30000.0

_CACHE = {}


def _build_nc(mm_dt_name="float32r"):
    import concourse.bacc as bacc
    import concourse.mybir as mybir
    import concourse.tile as tile
    from contextlib import ExitStack

    dt = mybir.dt
    mm_dt = getattr(dt, mm_dt_name)
    f32 = dt.float32
    bf16 = dt.bfloat16
    Exp = mybir.ActivationFunctionType.Exp

    nc = bacc.Bacc(None)
    xt = nc.declare_dram_parameter("xt", [D, N], f32, isOutput=False)
    xtq = nc.declare_dram_parameter("xtq", [D, NL], f32, isOutput=False)
    wq = nc.declare_dram_parameter("wq", [D, D], f32, isOutput=False)
    wk = nc.declare_dram_parameter("wk", [D, D], f32, isOutput=False)
    wv = nc.declare_dram_parameter("wv", [D, D], f32, isOutput=False)
    wo = nc.declare_dram_parameter("wo", [D, D], f32, isOutput=False)
    cst = nc.declare_dram_parameter("cst", [3, 128, 128], bf16, isOutput=False)
    o = nc.declare_dram_parameter("o", [NL, D], f32, isOutput=True)

    def r(ap):  # matmul-dtype view
        return ap.bitcast(mm_dt)

    with tile.TileContext(nc) as tc:
        with ExitStack() as es:
            persist = es.enter_context(tc.tile_pool(name="persist", bufs=1))
            kT = [persist.tile([128, N], f32, tag=f"kT{f}", name=f"kT{f}")
                  for f in range(FT)]
            den = persist.tile([H, NL], f32, tag="den", name="den")
            rec = persist.tile([H, NL], f32, tag="rec", name="rec")
            msk = persist.tile([128, 3 * 128], bf16, tag="msk", name="msk")
            step = msk[:, 0:128]
            dsel = [msk[:, 128 * (1 + p_):128 * (2 + p_)] for p_ in range(2)]
            apool = es.enter_context(tc.tile_pool(name="apool", bufs=1))
            aT = [apool.tile([64, NL], f32, tag=f"aT{h}", name=f"aT{h}")
                  for h in range(H)]
            for i3 in range(3):
                nc.sync.dma_start(out=msk[:, i3 * 128:(i3 + 1) * 128], in_=cst[i3])

            GF = 3   # feature tiles (pairs) per head group
            for grp in range(2):
                f0 = grp * GF
                with ExitStack() as ges:
                    qvp = ges.enter_context(
                        tc.tile_pool(name=f"qvp{grp}", bufs=1))
                    qT = [qvp.tile([128, NL], f32, tag=f"qT{f}", name=f"qT{f}")
                          for f in range(GF)]
                    VW = 2 * GF * (DH + 1)  # 390
                    vp_all = qvp.tile([128, JT * VW], f32, tag="vp", name="vp")
                    vp = [vp_all[:, j * VW:(j + 1) * VW] for j in range(JT)]

                    # ---- q projection (quarters of local rows) ----
                    with tc.tile_pool(name="wqp", bufs=1) as wqp, \
                         tc.tile_pool(name="xqp", bufs=2) as xqp, \
                         tc.tile_pool(name="pp1", bufs=3, space="PSUM") as pp1:
                        wq_a = wqp.tile([128, KC * 2 * GF * 64], f32, tag="wqa",
                                        name="wqa")
                        for k in range(KC):
                            nc.sync.dma_start(
                                out=r(wq_a[:, k * 384:(k + 1) * 384]),
                                in_=r(wq[k * 128:(k + 1) * 128,
                                       f0 * 128:(f0 + GF) * 128]))
                        for qtr in range(4):
                            xq_a = xqp.tile([128, KC * 256], f32, tag="xqa",
                                            name="xqa")
                            for k in range(KC):
                                nc.sync.dma_start(
                                    out=r(xq_a[:, k * 256:(k + 1) * 256]),
                                    in_=r(xtq[k * 128:(k + 1) * 128,
                                            qtr * 256:(qtr + 1) * 256]))
                            for f in range(GF):
                                ps = pp1.tile([128, 256], f32, tag="ps1",
                                              name="ps1")
                                for k in range(KC):
                                    nc.tensor.matmul(
                                        out=ps[:],
                                        lhsT=r(wq_a[:, k * 384 + f * 128:
                                                    k * 384 + (f + 1) * 128]),
                                        rhs=r(xq_a[:, k * 256:(k + 1) * 256]),
                                        start=(k == 0), stop=(k == KC - 1))
                                nc.vector.tensor_copy(
                                    r(qT[f][:, qtr * 256:(qtr + 1) * 256]), ps[:])

                    # ---- k projection (quarters of sequence) ----
                    with tc.tile_pool(name="wkp", bufs=1) as wkp, \
                         tc.tile_pool(name="xhp", bufs=2) as xhp, \
                         tc.tile_pool(name="pp2", bufs=3, space="PSUM") as pp2:
                        wk_a = wkp.tile([128, KC * GF * 128], f32, tag="wka",
                                        name="wka")
                        for k in range(KC):
                            nc.sync.dma_start(
                                out=r(wk_a[:, k * 384:(k + 1) * 384]),
                                in_=r(wk[k * 128:(k + 1) * 128,
                                       f0 * 128:(f0 + GF) * 128]))
                        for qtr in range(4):
                            c0 = qtr * 512
                            xh_a = xhp.tile([128, KC * 512], f32, tag="xha",
                                            name="xha")
                            for k in range(KC):
                                nc.sync.dma_start(
                                    out=r(xh_a[:, k * 512:(k + 1) * 512]),
                                    in_=r(xt[k * 128:(k + 1) * 128, c0:c0 + 512]))
                            for f in range(GF):
                                ps = pp2.tile([128, 512], f32, tag="ps2",
                                              name="ps2")
                                for k in range(KC):
                                    nc.tensor.matmul(
                                        out=ps[:],
                                        lhsT=r(wk_a[:, k * 384 + f * 128:
                                                    k * 384 + (f + 1) * 128]),
                                        rhs=r(xh_a[:, k * 512:(k + 1) * 512]),
                                        start=(k == 0), stop=(k == KC - 1))
                                nc.vector.tensor_copy(r(kT[f][:, c0:c0 + 512]),
                                                      ps[:])

                    # ---- v projection (quarters of sequence) ----
                    with tc.tile_pool(name="wvp", bufs=1) as wvp, \
                         tc.tile_pool(name="xhq", bufs=2) as xhq, \
                         tc.tile_pool(name="pp3", bufs=3, space="PSUM") as pp3:
                        wv_a = wvp.tile([128, KC * 2 * GF * 64], f32, tag="wva",
                                        name="wva")
                        for k in range(KC):
                            nc.sync.dma_start(
                                out=r(wv_a[:, k * 384:(k + 1) * 384]),
                                in_=r(wv[k * 128:(k + 1) * 128,
                                       f0 * 128:(f0 + GF) * 128]))
                        for qtr in range(4):
                            c0 = qtr * 512
                            xh_a = xhq.tile([128, KC * 512], f32, tag="xhb",
                                            name="xhb")
                            for k in range(KC):
                                nc.sync.dma_start(
                                    out=r(xh_a[:, k * 512:(k + 1) * 512]),
                                    in_=r(xt[k * 128:(k + 1) * 128, c0:c0 + 512]))
                            for rr in range(4):
                                jt = qtr * 4 + rr
                                ps = pp3.tile([128, 384], f32, tag="ps3",
                                              name="ps3")
                                for k in range(KC):
                                    nc.tensor.matmul(
                                        out=ps[:],
                                        lhsT=r(xh_a[:, k * 512 + rr * 128:
                                                    k * 512 + (rr + 1) * 128]),
                                        rhs=r(wv_a[:, k * 384:(k + 1) * 384]),
                                        start=(k == 0), stop=(k == KC - 1))
                                vv = vp[jt].rearrange("p (h c) -> p h c",
                                                      c=DH + 1)
                                nc.vector.tensor_copy(
                                    r(vv[:, :, 0:DH]),
                                    ps[:].rearrange("p (h c) -> p h c", c=DH))

                    with tc.tile_pool(name="on", bufs=1) as onp:
                        ones96 = onp.tile([128, JT * 2 * GF], f32, tag="on",
                                          name="ones96")
                        nc.vector.memset(ones96[:], 1.0)
                        vview = vp_all.rearrange("p (j c) -> p j c", c=DH + 1)
                        nc.vector.tensor_copy(
                            r(vview[:, :, DH:DH + 1]),
                            ones96[:].rearrange("p (a b) -> p a b", b=1))

                    # ---- attention ----
                    with tc.tile_pool(name="et", bufs=3) as etp, \
                         tc.tile_pool(name="ps4", bufs=2, space="PSUM") as ps4, \
                         tc.tile_pool(name="po4", bufs=1, space="PSUM") as po4:
                        for f in range(GF):
                            oT = [po4.tile([DH + 1, NL], f32, tag=f"oT{i}",
                                           name=f"oT{i}") for i in range(2)]
                            for jt in range(JT):
                                tmin = jt // 2
                                ic = NL - tmin * 128
                                for hh in range(2):
                                    hl = 2 * f + hh
                                    hg = 2 * (f0 + f) + hh
                                    hs = slice(hh * 64, hh * 64 + 64)
                                    ps = ps4.tile([128, NL], f32, tag="ps",
                                                  name="ps")
                                    nch = [(c, min(c + 512, ic))
                                           for c in range(0, ic, 512)]
                                    (n0, n1) = nch[0]
                                    nc.tensor.matmul(
                                        out=ps[:, n0:n1],
                                        lhsT=r(kT[f][hs, jt * 128:(jt + 1) * 128]),
                                        rhs=r(qT[f][hs, tmin * 128 + n0:
                                                    tmin * 128 + n1]),
                                        start=True, stop=False,
                                        skip_group_check=True)
                                    nc.tensor.matmul(
                                        out=ps[:, 0:128], lhsT=step,
                                        rhs=dsel[jt % 2], start=False, stop=True,
                                        skip_group_check=True)
                                    for (n0, n1) in nch[1:]:
                                        nc.tensor.matmul(
                                            out=ps[:, n0:n1],
                                            lhsT=r(kT[f][hs, jt * 128:(jt + 1) * 128]),
                                            rhs=r(qT[f][hs, tmin * 128 + n0:
                                                        tmin * 128 + n1]),
                                            start=True, stop=True,
                                            skip_group_check=True)
                                    et = etp.tile([128, NL], f32, tag="et",
                                                  name="et")
                                    nc.scalar.activation(
                                        out=r(et[:, 0:ic]), in_=ps[:, 0:ic],
                                        func=Exp, scale=0.125)
                                    for (n0, n1) in nch:
                                        nc.tensor.matmul(
                                            out=oT[hh][:, tmin * 128 + n0:
                                                       tmin * 128 + n1],
                                            lhsT=r(vp[jt][:, hl * (DH + 1):
                                                          (hl + 1) * (DH + 1)]),
                                            rhs=r(et[:, n0:n1]),
                                            start=(jt == 0), stop=(jt == JT - 1),
                                            skip_group_check=True)
                            for hh in range(2):
                                hg = 2 * (f0 + f) + hh
                                nc.vector.tensor_copy(r(aT[hg][:]), oT[hh][0:DH, :])
                                dtmp = etp.tile([65, NL], f32, tag="dtmp",
                                                name="dtmp")
                                nc.vector.tensor_copy(dtmp[64:65, :],
                                                      oT[hh][DH:DH + 1, :])
                                nc.sync.dma_start(out=den[hg:hg + 1, :],
                                                  in_=dtmp[64:65, :])

            # ---------------- normalize ----------------
            import concourse.bass as bass
            nc.vector.reciprocal(out=rec[:], in_=den[:])
            with tc.tile_pool(name="rb", bufs=3) as rbp, \
                 tc.tile_pool(name="dr", bufs=1, space="DRAM") as drp:
                recd = drp.tile([H, NL], f32, tag="recd", name="recd")
                nc.sync.dma_start(out=recd[:], in_=rec[:])
                for h in range(H):
                    rb = rbp.tile([64, NL], f32, tag="rb", name="rb")
                    src = recd[h:h + 1, :]
                    bcast = bass.AP(tensor=src.tensor, offset=src.offset,
                                    ap=[[0, 64]] + [list(a) for a in src.ap[1:]])
                    nc.gpsimd.dma_start(out=rb[:], in_=bcast)
                    nc.vector.tensor_mul(r(aT[h][:]), aT[h][:], rb[:])

            # ---------------- output projection ----------------
            with tc.tile_pool(name="wop", bufs=1) as wop, \
                 tc.tile_pool(name="osb", bufs=2) as osb, \
                 tc.tile_pool(name="pp5", bufs=2, space="PSUM") as pp5:
                wo_a = wop.tile([64, H * D], f32, tag="woa", name="woa")
                for h in range(H):
                    nc.sync.dma_start(out=r(wo_a[:, h * D:(h + 1) * D]),
                                      in_=r(wo[h * 64:(h + 1) * 64, :]))
                for isl in range(NL // 128):
                    ps = pp5.tile([128, D], f32, tag="ps5", name="ps5")
                    for h in range(H):
                        for (n0, n1) in ((0, 512), (512, 768)):
                            nc.tensor.matmul(
                                out=ps[:, n0:n1],
                                lhsT=r(aT[h][:, isl * 128:(isl + 1) * 128]),
                                rhs=r(wo_a[:, h * D + n0:h * D + n1]),
                                start=(h == 0), stop=(h == H - 1))
                    ot = osb.tile([128, D], f32, tag="ot", name="ot")
                    nc.vector.tensor_copy(ot[:], ps[:])
                    nc.sync.dma_start(out=o[isl * 128:(isl + 1) * 128, :],
                                      in_=ot[:])

    nc.finalize()
    return nc


def _mask_tiles(par):
    import ml_dtypes
    # step[r, jp] = 1 iff r <= jp;  D[jp, q] = NEG * [row(q) <= jp]
    step = np.tril(np.ones((128, 128), np.float32), 0).T
    d0 = np.zeros((128, 128), np.float32)
    d1 = np.zeros((128, 128), np.float32)
    for q in range(128):
        rr = 2 * q + par + 1          # mask iff jp > 2q+par
        if rr < 128:
            d0[rr, q] = NEG
        rr = 2 * q + par - 127        # mask iff jp + 128 > 2q+par
        if rr < 128:
            d1[max(rr, 0), q] = NEG
    return np.stack([step, d0, d1]).astype(ml_dtypes.bfloat16)


def _host_reference(x, mask, w_qkv, w_out):
    qkv = x.astype(np.float64) @ w_qkv.astype(np.float64)
    q, k, v = np.split(qkv, 3, axis=-1)

    def heads(t):
        return t.reshape(B, N, H, DH).transpose(0, 2, 1, 3)
    q, k, v = heads(q), heads(k), heads(v)
    s = np.einsum('bhqd,bhkd->bhqk', q, k) / np.sqrt(DH)
    s = np.where(np.asarray(mask).reshape(1, 1, N, N) == 0, -np.inf, s)
    s = s - s.max(-1, keepdims=True)
    e = np.exp(s)
    p = e / e.sum(-1, keepdims=True)
    out = np.einsum('bhqk,bhkd->bhqd', p, v)
    out = out.transpose(0, 2, 1, 3).reshape(B, N, D)
    return (out @ w_out.astype(np.float64)).astype(np.float32)


def kernel(x, mask, w_qkv, w_out):
    x = np.asarray(x)
    w_qkv = np.asarray(w_qkv)
    w_out = np.asarray(w_out)

    causal = np.array_equal(
        np.asarray(mask).reshape(N, N) != 0, np.tril(np.ones((N, N), bool)))
    if not causal:
        return _host_reference(x, mask, w_qkv, w_out)

    from concourse.bass_utils import run_bass_kernel_spmd
    if "nc" not in _CACHE:
        _CACHE["nc"] = _build_nc()
    nc = _CACHE["nc"]

    wq = np.ascontiguousarray(w_qkv[:, 0:D])
    wk = np.ascontiguousarray(w_qkv[:, D:2 * D])
    wv = np.ascontiguousarray(w_qkv[:, 2 * D:3 * D])
    wo = np.ascontiguousarray(w_out)
    csts = [_mask_tiles(0), _mask_tiles(1)]

    in_maps = []
    for c in range(8):
        b, par = c // 2, c % 2
        xb = x[b]
        in_maps.append({
            "xt": np.ascontiguousarray(xb.T),
            "xtq": np.ascontiguousarray(xb[par::2, :].T),
            "wq": wq, "wk": wk, "wv": wv, "wo": wo,
            "cst": csts[par],
        })
    res = run_bass_kernel_spmd(nc, in_maps, core_ids=list(range(8)),
                               **_CACHE.get("run_kwargs", {}))
    _CACHE["last_res"] = res
    out = np.empty((B, N, D), np.float32)
    for c in range(8):
        b, par = c // 2, c % 2
        out[b, par::2, :] = res.results[c]["o"]
    return out



# revision 5
# speedup vs baseline: 1.9059x; 1.9059x over previous
"""Causal multi-head attention (B=4, N=2048, D=768, H=12) on 8 TRN2 cores.

Sharding (per spec hint): data-parallel over batch (4) x tensor-parallel
over heads (2 groups of 6 heads). Core c handles batch c//2, heads
(c%2)*6 .. +6, over the FULL sequence. Host splits w_qkv/w_out
column/row-wise per head group and sums the two partial out-projection
results per batch (row-parallel reduction done host-side).

Per core, fully on-device, no collectives (all bf16 operands, f32 psum):
  qT/kT = Wq/k^T x^T   ([feat, seq] layout, 3 tiles of 2 heads each)
  v     = x Wv         ([seq, feat] layout + ones column for denoms)
  flash-style: per (head, 1024-query window, 128-key tile):
    s = kT_tile^T qT_win (PSUM f32); e = exp(s/8) -> bf16 SBUF
    diagonal tiles: e *= tril-mask (gpsimd)
    oT[65, win] += v_tile^T e  (65th row = softmax denominators)
  aT = oT[0:64] * (1/denom broadcast)   out = aT^T Wo  (partial, f32)
"""

import numpy as np

B, N, D, H = 4, 2048, 768, 12
DH = 64            # head dim
HPC = 6            # heads per core
FB = 3             # feature blocks (2 heads = 128 feats) per core
KC = 6             # contraction chunks (768 / 128)
JT = 16            # key tiles (2048 / 128)
QW = 1024          # query window
NQW = N // QW      # 2 windows
G = HPC * DH       # 384 features per core
VW = HPC * (DH + 1)  # 390 v-columns per key tile (with ones)

_CACHE = {}


def _build_nc(mm_dt_name="bfloat16"):
    import concourse.bacc as bacc
    import concourse.mybir as mybir
    import concourse.tile as tile
    import concourse.bass as cbass
    from contextlib import ExitStack

    dt = mybir.dt
    f32 = dt.float32
    bf16 = dt.bfloat16
    Exp = mybir.ActivationFunctionType.Exp

    nc = bacc.Bacc(None)
    xt = nc.declare_dram_parameter("xt", [D, N], bf16, isOutput=False)
    wq = nc.declare_dram_parameter("wq", [D, G], bf16, isOutput=False)
    wk = nc.declare_dram_parameter("wk", [D, G], bf16, isOutput=False)
    wv = nc.declare_dram_parameter("wv", [D, G], bf16, isOutput=False)
    wo = nc.declare_dram_parameter("wo", [G, D], bf16, isOutput=False)
    msk = nc.declare_dram_parameter("msk", [128, 128], bf16, isOutput=False)
    o = nc.declare_dram_parameter("o", [N, D], f32, isOutput=True)

    with tile.TileContext(nc) as tc:
        with ExitStack() as es:
            persist = es.enter_context(tc.tile_pool(name="persist", bufs=1))
            xts = persist.tile([128, KC * N], bf16, tag="xts", name="xts")
            wqs = persist.tile([128, KC * G], bf16, tag="wqs", name="wqs")
            wks = persist.tile([128, KC * G], bf16, tag="wks", name="wks")
            wvs = persist.tile([128, KC * G], bf16, tag="wvs", name="wvs")
            wos = persist.tile([128, FB * D], bf16, tag="wos", name="wos")
            qT = [persist.tile([128, N], bf16, tag=f"qT{f}", name=f"qT{f}")
                  for f in range(FB)]
            kT = [persist.tile([128, N], bf16, tag=f"kT{f}", name=f"kT{f}")
                  for f in range(FB)]
            aT = [persist.tile([128, N], bf16, tag=f"aT{f}", name=f"aT{f}")
                  for f in range(FB)]
            vsb = persist.tile([128, JT * VW], bf16, tag="vsb", name="vsb")
            mskt = persist.tile([128, 128], bf16, tag="mskt", name="mskt")

            nc.sync.dma_start(out=mskt[:], in_=msk[:, :])
            for c in range(KC):
                nc.sync.dma_start(out=xts[:, c * N:(c + 1) * N],
                                  in_=xt[c * 128:(c + 1) * 128, :])
            for (w_sb, w_dr) in ((wqs, wq), (wks, wk), (wvs, wv)):
                for c in range(KC):
                    nc.sync.dma_start(out=w_sb[:, c * G:(c + 1) * G],
                                      in_=w_dr[c * 128:(c + 1) * 128, :])
            for p in range(FB):
                nc.sync.dma_start(out=wos[:, p * D:(p + 1) * D],
                                  in_=wo[p * 128:(p + 1) * 128, :])

            # ---- v projection: v[keys, feats] + ones column ----
            vv_all = vsb.rearrange("p (j c) -> p j c", c=DH + 1)
            nc.vector.memset(vv_all[:, :, DH:DH + 1], 1.0)
            with tc.tile_pool(name="pv", bufs=2, space="PSUM") as pv:
                for kb in range(JT):
                    ps = pv.tile([128, G], f32, tag="psv", name="psv")
                    for c in range(KC):
                        nc.tensor.matmul(
                            out=ps[:],
                            lhsT=xts[:, c * N + kb * 128:c * N + (kb + 1) * 128],
                            rhs=wvs[:, c * G:(c + 1) * G],
                            start=(c == 0), stop=(c == KC - 1))
                    dst = vsb[:, kb * VW:(kb + 1) * VW].rearrange(
                        "p (h c) -> p h c", c=DH + 1)
                    nc.vector.tensor_copy(
                        dst[:, :, 0:DH],
                        ps[:].rearrange("p (h c) -> p h c", c=DH))

            # ---- q/k projections: [feat, seq] layout ----
            with tc.tile_pool(name="pqk", bufs=2, space="PSUM") as pqk:
                for (w_sb, dstT) in ((wqs, qT), (wks, kT)):
                    for fb in range(FB):
                        ps = pqk.tile([128, N], f32, tag="psqk", name="psqk")
                        for c in range(KC):
                            for qc in range(4):
                                nc.tensor.matmul(
                                    out=ps[:, qc * 512:(qc + 1) * 512],
                                    lhsT=w_sb[:, c * G + fb * 128:
                                              c * G + (fb + 1) * 128],
                                    rhs=xts[:, c * N + qc * 512:
                                            c * N + (qc + 1) * 512],
                                    start=(c == 0), stop=(c == KC - 1),
                                    skip_group_check=True)
                        nc.vector.tensor_copy(dstT[fb][:], ps[:])

            # ---- attention ----
            with tc.tile_pool(name="et", bufs=3) as etp, \
                 tc.tile_pool(name="pss", bufs=2, space="PSUM") as pss, \
                 tc.tile_pool(name="pso", bufs=2, space="PSUM") as pso, \
                 tc.tile_pool(name="dt", bufs=2) as dtp, \
                 tc.tile_pool(name="tb", bufs=2) as tbp, \
                 tc.tile_pool(name="rb", bufs=2) as rbp, \
                 tc.tile_pool(name="dr", bufs=1, space="DRAM") as drp:
                recd = drp.tile([HPC, N], f32, tag="recd", name="recd")
                for h in range(HPC):
                    fb, hh = h // 2, h % 2
                    hs = slice(hh * 64, hh * 64 + 64)
                    for qb in range(NQW):
                        ntile = 8 * qb + 8
                        qc0 = qb * QW
                        oT = pso.tile([128, QW], f32, tag="oT", name="oT")

                        def chunks(c0):
                            if c0 < 512:
                                return ((c0, 512), (512, QW))
                            return ((c0, QW),)

                        prev = None
                        for jt in range(ntile):
                            c0 = max(0, (jt - 8 * qb) * 128)
                            ps = pss.tile([128, QW], f32, tag="ps", name="ps")
                            for (a, b_) in chunks(c0):
                                nc.tensor.matmul(
                                    out=ps[:, a:b_],
                                    lhsT=kT[fb][hs, jt * 128:(jt + 1) * 128],
                                    rhs=qT[fb][hs, qc0 + a:qc0 + b_],
                                    start=True, stop=True,
                                    skip_group_check=True)
                            if prev is not None:
                                pjt, pet, pc0 = prev
                                vsl = vsb[:, pjt * VW + h * (DH + 1):
                                          pjt * VW + (h + 1) * (DH + 1)]
                                for (a, b_) in chunks(pc0):
                                    nc.tensor.matmul(
                                        out=oT[0:DH + 1, a:b_],
                                        lhsT=vsl, rhs=pet[:, a:b_],
                                        start=(pjt == 0), stop=False,
                                        skip_group_check=True)
                            et = etp.tile([128, QW], bf16, tag="et", name="et")
                            nc.scalar.activation(out=et[:, c0:QW],
                                                 in_=ps[:, c0:QW],
                                                 func=Exp, scale=0.125)
                            if jt >= 8 * qb:
                                nc.gpsimd.tensor_mul(
                                    et[:, c0:c0 + 128],
                                    et[:, c0:c0 + 128], mskt[:])
                            prev = (jt, et, c0)
                        pjt, pet, pc0 = prev
                        vsl = vsb[:, pjt * VW + h * (DH + 1):
                                  pjt * VW + (h + 1) * (DH + 1)]
                        for (a, b_) in chunks(pc0):
                            nc.tensor.matmul(
                                out=oT[0:DH + 1, a:b_],
                                lhsT=vsl, rhs=pet[:, a:b_],
                                start=False, stop=True,
                                skip_group_check=True)

                        # evacuate window: features -> aT, 1/denom -> recd
                        dtt = dtp.tile([DH + 1, QW], f32, tag="dt", name="dt")
                        if hh == 0:
                            nc.vector.tensor_copy(
                                aT[fb][0:DH, qc0:qc0 + QW], oT[0:DH, :])
                        else:
                            tbt = tbp.tile([DH, QW], bf16, tag="tb", name="tb")
                            nc.vector.tensor_copy(tbt[:], oT[0:DH, :])
                            nc.sync.dma_start(
                                out=aT[fb][DH:2 * DH, qc0:qc0 + QW],
                                in_=tbt[:])
                        nc.vector.reciprocal(out=dtt[DH:DH + 1, :],
                                             in_=oT[DH:DH + 1, :])
                        nc.sync.dma_start(out=recd[h:h + 1, qc0:qc0 + QW],
                                          in_=dtt[DH:DH + 1, :])

                    # normalize this head (pipelines behind next head)
                    rbt = rbp.tile([128, N], f32, tag="rb", name="rb")
                    src = recd[h:h + 1, :]
                    bcast = cbass.AP(tensor=src.tensor, offset=src.offset,
                                     ap=[[0, 64]] + [list(a) for a in src.ap[1:]])
                    nc.gpsimd.dma_start(out=rbt[hs, :], in_=bcast)
                    nc.vector.tensor_mul(aT[fb][hs, :], aT[fb][hs, :],
                                         rbt[hs, :])

            # ---- output projection (partial; host sums core pairs) ----
            with tc.tile_pool(name="po", bufs=2, space="PSUM") as pop, \
                 tc.tile_pool(name="ob", bufs=3) as obp:
                for ib in range(N // 128):
                    ps = pop.tile([128, D], f32, tag="pso2", name="pso2")
                    for p in range(FB):
                        for (a, b_) in ((0, 512), (512, D)):
                            nc.tensor.matmul(
                                out=ps[:, a:b_],
                                lhsT=aT[p][:, ib * 128:(ib + 1) * 128],
                                rhs=wos[:, p * D + a:p * D + b_],
                                start=(p == 0), stop=(p == FB - 1),
                                skip_group_check=True)
                    ot = obp.tile([128, D], f32, tag="ot", name="ot")
                    nc.vector.tensor_copy(ot[:], ps[:])
                    nc.sync.dma_start(out=o[ib * 128:(ib + 1) * 128, :],
                                      in_=ot[:])

    nc.finalize()
    return nc


def _host_reference(x, mask, w_qkv, w_out):
    qkv = x.astype(np.float64) @ w_qkv.astype(np.float64)
    q, k, v = np.split(qkv, 3, axis=-1)

    def heads(t):
        return t.reshape(B, N, H, DH).transpose(0, 2, 1, 3)
    q, k, v = heads(q), heads(k), heads(v)
    s = np.einsum('bhqd,bhkd->bhqk', q, k) / np.sqrt(DH)
    s = np.where(np.asarray(mask).reshape(1, 1, N, N) == 0, -np.inf, s)
    s = s - s.max(-1, keepdims=True)
    e = np.exp(s)
    p = e / e.sum(-1, keepdims=True)
    out = np.einsum('bhqk,bhkd->bhqd', p, v)
    out = out.transpose(0, 2, 1, 3).reshape(B, N, D)
    return (out @ w_out.astype(np.float64)).astype(np.float32)


def kernel(x, mask, w_qkv, w_out):
    import ml_dtypes
    bf = ml_dtypes.bfloat16
    x = np.asarray(x)
    w_qkv = np.asarray(w_qkv)
    w_out = np.asarray(w_out)

    causal = np.array_equal(
        np.asarray(mask).reshape(N, N) != 0, np.tril(np.ones((N, N), bool)))
    if not causal:
        return _host_reference(x, mask, w_qkv, w_out)

    from concourse.bass_utils import run_bass_kernel_spmd
    if "nc" not in _CACHE:
        _CACHE["nc"] = _build_nc()
    nc = _CACHE["nc"]

    msk_np = np.triu(np.ones((128, 128), np.float32)).astype(bf)
    in_maps = []
    for c in range(8):
        b, g = c // 2, c % 2
        in_maps.append({
            "xt": np.ascontiguousarray(x[b].T).astype(bf),
            "wq": np.ascontiguousarray(
                w_qkv[:, g * G:(g + 1) * G]).astype(bf),
            "wk": np.ascontiguousarray(
                w_qkv[:, D + g * G:D + (g + 1) * G]).astype(bf),
            "wv": np.ascontiguousarray(
                w_qkv[:, 2 * D + g * G:2 * D + (g + 1) * G]).astype(bf),
            "wo": np.ascontiguousarray(
                w_out[g * G:(g + 1) * G, :]).astype(bf),
            "msk": msk_np,
        })
    res = run_bass_kernel_spmd(nc, in_maps, core_ids=list(range(8)),
                               **_CACHE.get("run_kwargs", {}))
    _CACHE["last_res"] = res
    out = np.empty((B, N, D), np.float32)
    for b in range(B):
        out[b] = res.results[2 * b]["o"]
        out[b] += res.results[2 * b + 1]["o"]
    return out
